# revision 16
# baseline (speedup 1.0000x reference)
"""Trainium2 Bass kernel: out = 2 * cummax_W(cummax_H(x)) for x [16,256,128,128] f32.

Precision: gate is rel_err < 2e-2; device works on xb = bf16(2*x) (host
downcast; x2 folded into the input -- exact since max/x2 commute and bf16*2 is
exact). Only error is the input rounding (~2^-9 relative).

The DVE scan (tensor_tensor_scan) runs at 2 cyc/elem and is the bottleneck;
TT-max on aligned contiguous bf16 runs at 0.5 cyc/elem. BOTH passes use a
pair-trick: pair-combine adjacent elements (0.5 cyc/elem), scan only the pair
maxima (half the elements), rebuild even outputs with one more TT-max (0.5);
odd outputs are the scan result itself. All DVE combine/fix APs keep the
2x_1P mode (even strides, 4B-aligned starts, contiguous runs); each pair scan
writes at +1 into a 66-per-slice padded layout so the fix window starts on an
even element and sees NEG at slice boundaries.

W pass: the host pre-splits W into even|odd blocks, so the combine reads two
contiguous blocks. The fix writes the even results straight into the merged
transpose-input tile; the odd results (the Z run) are merged next to them by
an SBUF->SBUF DMA copy (gpsimd ring; SBUF descriptors have no small-run
penalty), keeping PE transposes full 128x128.

H pass: the scalar engine deinterleaves h-even/h-odd while staging PSUM->SBUF
(strided reads there, off the critical path). Even-h results and odd-h (Z)
results are stored as two DMA streams into a [w', s, hE|hO] DRAM layout; the
host re-interleaves h, inverse-permutes w', and upcasts.
"""

from contextlib import ExitStack

import numpy as np

import concourse.bass as bass
import concourse.tile as tile
from concourse import bacc, mybir
from concourse.bass_utils import run_bass_kernel_spmd
from concourse.masks import make_identity

N_CORES = 8
B, C, H, W = 16, 256, 128, 128
S = (B // N_CORES) * C  # 512 slices per core
NEG = -3.0e38

F32 = mybir.dt.float32
BF16 = mybir.dt.bfloat16

LAST_RESULTS = None


def build_nc(n_slices: int = S, g: int = 16, bufs: int = 6, taper: int = 2) -> bass.Bass:
    nc = bacc.Bacc(None, target_bir_lowering=False)
    # h-major input with W deinterleaved: x[h, s*W + (wE|wO)]
    # output o[w', s*128 + (hE|hO)], w' = (evens | odds)
    x = nc.declare_dram_parameter("x", [H, n_slices * W], BF16, isOutput=False)
    o = nc.declare_dram_parameter("o", [W, n_slices * H], BF16, isOutput=True)

    gs = g // 2
    chunks = []
    pos = 0
    for _ in range(taper):
        chunks.append((pos, gs))
        pos += gs
    tail = n_slices - taper * gs
    while pos < tail:
        chunks.append((pos, g))
        pos += g
    for _ in range(taper):
        chunks.append((pos, gs))
        pos += gs
    assert pos == n_slices

    with ExitStack() as ctx:
        tc = ctx.enter_context(tile.TileContext(nc))
        consts = ctx.enter_context(tc.tile_pool(name="consts", bufs=1))
        ident = consts.tile([128, 128], BF16)
        make_identity(nc, ident)
        # pair-scan bias over padded m layout: NEG at each slice's first pad
        bias_m = consts.tile([128, g * 66], BF16)
        nc.vector.memset(bias_m, 0.0)
        for gi in range(g):
            nc.vector.memset(bias_m[:, gi * 66 : gi * 66 + 1], NEG)

        xpool = ctx.enter_context(tc.tile_pool(name="xt", bufs=bufs))
        apool = ctx.enter_context(tc.tile_pool(name="at", bufs=bufs))
        epool = ctx.enter_context(tc.tile_pool(name="be", bufs=bufs))
        opool = ctx.enter_context(tc.tile_pool(name="bo", bufs=bufs))
        mpool = ctx.enter_context(tc.tile_pool(name="mt", bufs=2))
        zwpool = ctx.enter_context(tc.tile_pool(name="zw", bufs=bufs))
        zhpool = ctx.enter_context(tc.tile_pool(name="zh", bufs=bufs))
        rpool = ctx.enter_context(tc.tile_pool(name="rt", bufs=bufs))
        pa_pool = ctx.enter_context(tc.tile_pool(name="pa", bufs=6, space="PSUM"))

        xv = x.ap()
        ov = o.ap()

        for ci, (s0, gc) in enumerate(chunks):
            fw = gc * W
            hw = gc * 64
            xt = xpool.tile([128, fw], BF16, tag="xt")
            nc.sync.dma_start(out=xt[:], in_=xv[:, s0 * W : s0 * W + fw])
            xts = xt[:].rearrange("p (s e) -> p s e", s=gc)  # e=128: wE|wO

            # --- W pass (pair trick) ---
            mtw = mpool.tile([128, g * 66], BF16, tag="mtw")
            mtwv = mtw[:, : gc * 66].rearrange("p (s e) -> p s e", s=gc)
            if ci < 2:
                mf = mtw[:].rearrange("p (s e) -> p s e", s=g)
                nc.vector.memset(mf[:, :, 0:2], NEG)
            nc.vector.tensor_tensor(
                mtwv[:, :, 2:66], xts[:, :, 0:64], xts[:, :, 64:128],
                mybir.AluOpType.max,
            )
            zw = zwpool.tile([128, gc * 66 + 4], BF16, tag="zw")
            nc.vector.tensor_tensor_scan(
                zw[:, 1 : gc * 66 + 1], bias_m[:, : gc * 66], mtw[:, : gc * 66],
                0.0, mybir.AluOpType.add, mybir.AluOpType.max,
            )
            zwv = zw[:, : gc * 66].rearrange("p (s e) -> p s e", s=gc)
            # merged transpose input: per slice [R_evens(64) | Z(64)]
            at = apool.tile([128, fw], BF16, tag="at")
            ats = at[:].rearrange("p (s e) -> p s e", s=gc)
            nc.vector.tensor_tensor(
                ats[:, :, 0:64], zwv[:, :, 2:66], xts[:, :, 0:64],
                mybir.AluOpType.max,
            )
            zwz = zw[:, 3 : 3 + gc * 66].rearrange("p (s e) -> p s e", s=gc)
            nc.gpsimd.dma_start(out=ats[:, :, 64:128], in_=zwz[:, :, 0:64])

            # --- transpose + deinterleaved scalar staging ---
            btE = epool.tile([128, hw], BF16, tag="be")
            btO = opool.tile([128, hw], BF16, tag="bo")
            btEv = btE[:].rearrange("p (s e) -> p s e", s=gc)
            btOv = btO[:].rearrange("p (s e) -> p s e", s=gc)
            for hb in range(gc // 8):
                pa = pa_pool.tile([128, 1024], BF16, tag="pa")
                for j in range(8):
                    s = hb * 8 + j
                    nc.tensor.transpose(
                        pa[:, j * 128 : (j + 1) * 128],
                        at[:, s * 128 : (s + 1) * 128],
                        ident[:],
                    )
                pav = pa[:].rearrange("p (s hj hb) -> p s hj hb", s=8, hb=2)
                nc.scalar.copy(btEv[:, hb * 8 : (hb + 1) * 8], pav[:, :, :, 0])
                nc.scalar.copy(btOv[:, hb * 8 : (hb + 1) * 8], pav[:, :, :, 1])

            # --- H pass (pair trick) ---
            mth = mpool.tile([128, g * 66], BF16, tag="mth")
            mthv = mth[:, : gc * 66].rearrange("p (s e) -> p s e", s=gc)
            if ci < 2:
                mf = mth[:].rearrange("p (s e) -> p s e", s=g)
                nc.vector.memset(mf[:, :, 0:2], NEG)
            nc.vector.tensor_tensor(
                mthv[:, :, 2:66], btEv[:], btOv[:], mybir.AluOpType.max
            )
            zh = zhpool.tile([128, gc * 66 + 4], BF16, tag="zh")
            nc.vector.tensor_tensor_scan(
                zh[:, 1 : gc * 66 + 1], bias_m[:, : gc * 66], mth[:, : gc * 66],
                0.0, mybir.AluOpType.add, mybir.AluOpType.max,
            )
            zhv = zh[:, : gc * 66].rearrange("p (s e) -> p s e", s=gc)
            rt = rpool.tile([128, hw], BF16, tag="rt")
            rts = rt[:].rearrange("p (s e) -> p s e", s=gc)
            nc.vector.tensor_tensor(
                rts[:], zhv[:, :, 2:66], btEv[:], mybir.AluOpType.max
            )
            # stores: evens from rt, odds (Z runs) from zh
            ovv = ov[:, s0 * H : s0 * H + fw].rearrange("p (s e) -> p s e", s=gc)
            nc.scalar.dma_start(out=ovv[:, :, 0:64], in_=rts[:])
            zhz = zh[:, 3 : 3 + gc * 66].rearrange("p (s e) -> p s e", s=gc)
            nc.gpsimd.dma_start(out=ovv[:, :, 64:128], in_=zhz[:, :, 0:64])
    nc.finalize()
    return nc


def kernel(x: np.ndarray) -> np.ndarray:
    global LAST_RESULTS
    import ml_dtypes

    assert x.shape == (B, C, H, W)
    xb = (np.asarray(x, dtype=np.float32) * 2.0).astype(ml_dtypes.bfloat16)
    xs = xb.reshape(N_CORES, S, H, W)
    # deinterleave W: per slice [evens | odds]
    xd = np.concatenate([xs[..., 0::2], xs[..., 1::2]], axis=-1)
    in_maps = [
        {"x": np.ascontiguousarray(xd[i].transpose(1, 0, 2)).reshape(H, S * W)}
        for i in range(N_CORES)
    ]
    nc = build_nc(S, g=16, bufs=6, taper=2)
    res = run_bass_kernel_spmd(nc, in_maps, core_ids=list(range(N_CORES)))
    LAST_RESULTS = res
    # o[w', s*128 + (hE|hO)]; w' 0..63 -> w=2k, 64..127 -> w=2k+1
    out = np.empty((N_CORES, S, H, W), dtype=np.float32)
    for i in range(N_CORES):
        oi = np.asarray(res.results[i]["o"]).reshape(W, S, 2, 64).astype(np.float32)
        t = np.empty((S, H, W), dtype=np.float32)
        t[:, 0::2, 0::2] = oi[0:64, :, 0, :].transpose(1, 2, 0)
        t[:, 1::2, 0::2] = oi[0:64, :, 1, :].transpose(1, 2, 0)
        t[:, 0::2, 1::2] = oi[64:128, :, 0, :].transpose(1, 2, 0)
        t[:, 1::2, 1::2] = oi[64:128, :, 1, :].transpose(1, 2, 0)
        out[i] = t
    return out.reshape(B, C, H, W)


# revision 18
# speedup vs baseline: 1.0580x; 1.0580x over previous
"""Trainium2 Bass kernel: out = 2 * cummax_W(cummax_H(x)) for x [16,256,128,128] f32.

Precision: gate is rel_err < 2e-2; device works on xb = bf16(2*x) (host
downcast; x2 folded into the input -- exact since max/x2 commute and bf16*2 is
exact). Only error is the input rounding (~2^-9 relative).

The DVE scan (tensor_tensor_scan) runs at 2 cyc/elem and is the bottleneck;
TT-max on aligned contiguous bf16 runs at 0.5 cyc/elem. The H pass therefore
uses a pair-trick: pair-combine adjacent h rows (0.5), scan only the pair
maxima (half the elements), and rebuild even-h outputs with one more TT-max
(0.5) -- odd-h outputs are the scan result itself. The h-even/odd split is
produced by the scalar engine while staging PSUM->SBUF (strided reads there
are off the critical path). All DVE combine/fix APs keep the 2x_1P mode:
even strides, 4B-aligned starts, contiguous runs; the scan output lands at +1
in a 66-per-slice padded layout so the fix window starts even and sees NEG at
slice boundaries.

Per core (512 slices), per supertile of g=16 slices:
  1. DMA load xt [p=h, f=(s,w)] bf16 (host h-major; 4KB descriptors).
  2. DVE W-scan: segmented cummax (bias NEG at each slice's first column).
  3. PE transposes -> PSUM bf16 [p=w, f=(s,h)], 8 slices per bank tile.
  4. Scalar: deinterleaved staging PSUM->SBUF: btE (h even), btO (h odd).
  5. DVE: combine m=max(btE,btO) into padded mt; segmented pair scan -> zt@+1;
     fix rtH = max(zt window, btE).
  6. Stores: rtH (h-even results) and zt Z-runs (h-odd results) to a
     [w, s, hE|hO] DRAM layout; host re-interleaves and upcasts.
"""

from contextlib import ExitStack

import numpy as np

import concourse.bass as bass
import concourse.tile as tile
from concourse import bacc, mybir
from concourse.bass_utils import run_bass_kernel_spmd
from concourse.masks import make_identity

N_CORES = 8
B, C, H, W = 16, 256, 128, 128
S = (B // N_CORES) * C  # 512 slices per core
NEG = -3.0e38

F32 = mybir.dt.float32
BF16 = mybir.dt.bfloat16

LAST_RESULTS = None


def build_nc(n_slices: int = S, g: int = 16, bufs: int = 6, taper: int = 2) -> bass.Bass:
    nc = bacc.Bacc(None, target_bir_lowering=False)
    # h-major input: x[h, s*W + w]; output o[w, s*128 + (hE|hO)]
    x = nc.declare_dram_parameter("x", [H, n_slices * W], BF16, isOutput=False)
    o = nc.declare_dram_parameter("o", [W, n_slices * H], BF16, isOutput=True)

    gs = g // 2
    chunks = []
    pos = 0
    for _ in range(taper):
        chunks.append((pos, gs))
        pos += gs
    tail = n_slices - taper * gs
    while pos < tail:
        chunks.append((pos, g))
        pos += g
    for _ in range(taper):
        chunks.append((pos, gs))
        pos += gs
    assert pos == n_slices

    with ExitStack() as ctx:
        tc = ctx.enter_context(tile.TileContext(nc))
        consts = ctx.enter_context(tc.tile_pool(name="consts", bufs=1))
        ident = consts.tile([128, 128], BF16)
        make_identity(nc, ident)
        # W-scan bias over (s,w): NEG at each slice's first column
        bias = consts.tile([128, g * W], BF16)
        nc.vector.memset(bias, 0.0)
        for gi in range(g):
            nc.vector.memset(bias[:, gi * W : gi * W + 1], NEG)
        # pair-scan bias over padded m layout: NEG at each slice's first pad
        bias_m = consts.tile([128, g * 66], BF16)
        nc.vector.memset(bias_m, 0.0)
        for gi in range(g):
            nc.vector.memset(bias_m[:, gi * 66 : gi * 66 + 1], NEG)

        xpool = ctx.enter_context(tc.tile_pool(name="xt", bufs=bufs))
        apool = ctx.enter_context(tc.tile_pool(name="at", bufs=bufs))
        epool = ctx.enter_context(tc.tile_pool(name="be", bufs=bufs))
        opool = ctx.enter_context(tc.tile_pool(name="bo", bufs=bufs))
        mpool = ctx.enter_context(tc.tile_pool(name="mt", bufs=3))
        zpool = ctx.enter_context(tc.tile_pool(name="zt", bufs=bufs))
        rpool = ctx.enter_context(tc.tile_pool(name="rt", bufs=bufs))
        pa_pool = ctx.enter_context(tc.tile_pool(name="pa", bufs=6, space="PSUM"))

        xv = x.ap()
        ov = o.ap()

        for ci, (s0, gc) in enumerate(chunks):
            fw = gc * W
            xt = xpool.tile([128, fw], BF16, tag="xt")
            nc.sync.dma_start(out=xt[:], in_=xv[:, s0 * W : s0 * W + fw])
            at = apool.tile([128, fw], BF16, tag="at")
            nc.vector.tensor_tensor_scan(
                at[:], bias[:, :fw], xt[:], 0.0,
                mybir.AluOpType.add, mybir.AluOpType.max,
            )
            # transposes + deinterleaved scalar staging
            hw = gc * 64
            btE = epool.tile([128, hw], BF16, tag="be")
            btO = opool.tile([128, hw], BF16, tag="bo")
            btEv = btE[:].rearrange("p (s e) -> p s e", s=gc)
            btOv = btO[:].rearrange("p (s e) -> p s e", s=gc)
            for hb in range(gc // 8):
                pa = pa_pool.tile([128, 1024], BF16, tag="pa")
                for j in range(8):
                    s = hb * 8 + j
                    nc.tensor.transpose(
                        pa[:, j * 128 : (j + 1) * 128],
                        at[:, s * 128 : (s + 1) * 128],
                        ident[:],
                    )
                pav = pa[:].rearrange("p (s hj hb) -> p s hj hb", s=8, hb=2)
                nc.scalar.copy(btEv[:, hb * 8 : (hb + 1) * 8], pav[:, :, :, 0])
                nc.scalar.copy(btOv[:, hb * 8 : (hb + 1) * 8], pav[:, :, :, 1])
            # pair combine into padded m layout (full-width tile so the NEG
            # pads persist across the pool's buffer rotation)
            mt = mpool.tile([128, g * 66], BF16, tag="mt")
            mts = mt[:, : gc * 66].rearrange("p (s e) -> p s e", s=gc)
            if ci < 3:
                mfull = mt[:].rearrange("p (s e) -> p s e", s=g)
                nc.vector.memset(mfull[:, :, 0:2], NEG)
            nc.vector.tensor_tensor(
                mts[:, :, 2:66], btEv[:], btOv[:], mybir.AluOpType.max
            )
            # segmented pair scan, written at +1 (Z_k at 66s+3+k)
            zt = zpool.tile([128, gc * 66 + 4], BF16, tag="zt")
            nc.vector.tensor_tensor_scan(
                zt[:, 1 : gc * 66 + 1], bias_m[:, : gc * 66], mt[:, : gc * 66], 0.0,
                mybir.AluOpType.add, mybir.AluOpType.max,
            )
            zts = zt[:, : gc * 66].rearrange("p (s e) -> p s e", s=gc)
            # fix: even-h outputs R_2k = max(Z_{k-1}, E_k); window starts even
            rt = rpool.tile([128, hw], BF16, tag="rt")
            rts = rt[:].rearrange("p (s e) -> p s e", s=gc)
            nc.vector.tensor_tensor(
                rts[:], zts[:, :, 2:66], btEv[:], mybir.AluOpType.max
            )
            # stores: evens from rt, odds (Z runs) from zt
            ovv = ov[:, s0 * H : s0 * H + fw].rearrange("p (s e) -> p s e", s=gc)
            nc.scalar.dma_start(out=ovv[:, :, 0:64], in_=rts[:])
            ztz = zt[:, 3 : 3 + gc * 66].rearrange("p (s e) -> p s e", s=gc)
            nc.gpsimd.dma_start(out=ovv[:, :, 64:128], in_=ztz[:, :, 0:64])
    nc.finalize()
    return nc


def kernel(x: np.ndarray) -> np.ndarray:
    global LAST_RESULTS
    import ml_dtypes

    assert x.shape == (B, C, H, W)
    xb = (np.asarray(x, dtype=np.float32) * 2.0).astype(ml_dtypes.bfloat16)
    xs = xb.reshape(N_CORES, S, H, W)
    in_maps = [
        {"x": np.ascontiguousarray(xs[i].transpose(1, 0, 2)).reshape(H, S * W)}
        for i in range(N_CORES)
    ]
    nc = build_nc(S, g=16, bufs=6, taper=4)
    res = run_bass_kernel_spmd(nc, in_maps, core_ids=list(range(N_CORES)))
    LAST_RESULTS = res
    # o[w, s*128 + (hE|hO)]: out[s, 2j+b, w] = o[w, s, b, j]
    out = np.empty((N_CORES, S, H, W), dtype=np.float32)
    for i in range(N_CORES):
        oi = np.asarray(res.results[i]["o"]).reshape(W, S, 2, 64).astype(np.float32)
        out[i, :, 0::2, :] = oi[:, :, 0, :].transpose(1, 2, 0)
        out[i, :, 1::2, :] = oi[:, :, 1, :].transpose(1, 2, 0)
    return out.reshape(B, C, H, W)


# revision 19
# speedup vs baseline: 1.0658x; 1.0073x over previous
"""Trainium2 Bass kernel: out = 2 * cummax_W(cummax_H(x)) for x [16,256,128,128] f32.

Precision: gate is rel_err < 2e-2; device works on xb = bf16(2*x) (host
downcast; x2 folded into the input -- exact since max/x2 commute and bf16*2 is
exact). Only error is the input rounding (~2^-9 relative).

The DVE scan (tensor_tensor_scan) runs at 2 cyc/elem and is the bottleneck;
TT-max on aligned contiguous bf16 runs at 0.5 cyc/elem. The H pass therefore
uses a pair-trick: pair-combine adjacent h rows (0.5), scan only the pair
maxima (half the elements), and rebuild even-h outputs with one more TT-max
(0.5) -- odd-h outputs are the scan result itself. The h-even/odd split is
produced by the scalar engine while staging PSUM->SBUF (strided reads there
are off the critical path). All DVE combine/fix APs keep the 2x_1P mode:
even strides, 4B-aligned starts, contiguous runs; the scan output lands at +1
in a 66-per-slice padded layout so the fix window starts even and sees NEG at
slice boundaries.

Per core (512 slices), per supertile of g=16 slices:
  1. DMA load xt [p=h, f=(s,w)] bf16 (host h-major; 4KB descriptors).
  2. DVE W-scan: segmented cummax (bias NEG at each slice's first column).
  3. PE transposes -> PSUM bf16 [p=w, f=(s,h)], 8 slices per bank tile.
  4. Scalar: deinterleaved staging PSUM->SBUF: btE (h even), btO (h odd).
  5. DVE: combine m=max(btE,btO) into padded mt; segmented pair scan -> zt@+1;
     fix rtH = max(zt window, btE).
  6. Stores: rtH (h-even results) and zt Z-runs (h-odd results) to a
     [w, s, hE|hO] DRAM layout; host re-interleaves and upcasts.
"""

from contextlib import ExitStack

import numpy as np

import concourse.bass as bass
import concourse.tile as tile
from concourse import bacc, mybir
from concourse.bass_utils import run_bass_kernel_spmd
from concourse.masks import make_identity

N_CORES = 8
B, C, H, W = 16, 256, 128, 128
S = (B // N_CORES) * C  # 512 slices per core
NEG = -3.0e38

F32 = mybir.dt.float32
BF16 = mybir.dt.bfloat16

LAST_RESULTS = None


def build_nc(n_slices: int = S, g: int = 16, bufs: int = 6, taper: int = 2) -> bass.Bass:
    nc = bacc.Bacc(None, target_bir_lowering=False)
    # h-major input: x[h, s*W + w]; output o[w, s*128 + (hE|hO)]
    x = nc.declare_dram_parameter("x", [H, n_slices * W], BF16, isOutput=False)
    o = nc.declare_dram_parameter("o", [W, n_slices * H], BF16, isOutput=True)

    gs = g // 2
    chunks = []
    pos = 0
    for _ in range(taper):
        chunks.append((pos, gs))
        pos += gs
    tail = n_slices - taper * gs
    while pos < tail:
        chunks.append((pos, g))
        pos += g
    for _ in range(taper):
        chunks.append((pos, gs))
        pos += gs
    assert pos == n_slices

    with ExitStack() as ctx:
        tc = ctx.enter_context(tile.TileContext(nc))
        consts = ctx.enter_context(tc.tile_pool(name="consts", bufs=1))
        ident = consts.tile([128, 128], BF16)
        make_identity(nc, ident)
        # W-scan bias over (s,w): NEG at each slice's first column
        bias = consts.tile([128, g * W], BF16)
        nc.vector.memset(bias, 0.0)
        for gi in range(g):
            nc.vector.memset(bias[:, gi * W : gi * W + 1], NEG)
        # pair-scan bias over padded m layout: NEG at each slice's first pad
        bias_m = consts.tile([128, g * 66], BF16)
        nc.vector.memset(bias_m, 0.0)
        for gi in range(g):
            nc.vector.memset(bias_m[:, gi * 66 : gi * 66 + 1], NEG)

        xpool = ctx.enter_context(tc.tile_pool(name="xt", bufs=bufs))
        apool = ctx.enter_context(tc.tile_pool(name="at", bufs=bufs))
        epool = ctx.enter_context(tc.tile_pool(name="be", bufs=bufs))
        opool = ctx.enter_context(tc.tile_pool(name="bo", bufs=bufs))
        mpool = ctx.enter_context(tc.tile_pool(name="mt", bufs=3))
        zpool = ctx.enter_context(tc.tile_pool(name="zt", bufs=bufs))
        rpool = ctx.enter_context(tc.tile_pool(name="rt", bufs=bufs))
        pa_pool = ctx.enter_context(tc.tile_pool(name="pa", bufs=6, space="PSUM"))

        xv = x.ap()
        ov = o.ap()

        for ci, (s0, gc) in enumerate(chunks):
            fw = gc * W
            xt = xpool.tile([128, fw], BF16, tag="xt")
            nc.sync.dma_start(out=xt[:], in_=xv[:, s0 * W : s0 * W + fw])
            at = apool.tile([128, fw], BF16, tag="at")
            nc.vector.tensor_tensor_scan(
                at[:], bias[:, :fw], xt[:], 0.0,
                mybir.AluOpType.add, mybir.AluOpType.max,
            )
            # transposes + deinterleaved scalar staging
            hw = gc * 64
            btE = epool.tile([128, hw], BF16, tag="be")
            btO = opool.tile([128, hw], BF16, tag="bo")
            btEv = btE[:].rearrange("p (s e) -> p s e", s=gc)
            btOv = btO[:].rearrange("p (s e) -> p s e", s=gc)
            for hb in range(gc // 8):
                pa = pa_pool.tile([128, 1024], BF16, tag="pa")
                for j in range(8):
                    s = hb * 8 + j
                    nc.tensor.transpose(
                        pa[:, j * 128 : (j + 1) * 128],
                        at[:, s * 128 : (s + 1) * 128],
                        ident[:],
                    )
                pav = pa[:].rearrange("p (s hj hb) -> p s hj hb", s=8, hb=2)
                nc.scalar.copy(btEv[:, hb * 8 : (hb + 1) * 8], pav[:, :, :, 0])
                nc.scalar.copy(btOv[:, hb * 8 : (hb + 1) * 8], pav[:, :, :, 1])
            # pair combine into padded m layout (full-width tile so the NEG
            # pads persist across the pool's buffer rotation)
            mt = mpool.tile([128, g * 66], BF16, tag="mt")
            mts = mt[:, : gc * 66].rearrange("p (s e) -> p s e", s=gc)
            if ci < 3:
                mfull = mt[:].rearrange("p (s e) -> p s e", s=g)
                nc.vector.memset(mfull[:, :, 0:2], NEG)
            nc.vector.tensor_tensor(
                mts[:, :, 2:66], btEv[:], btOv[:], mybir.AluOpType.max
            )
            # segmented pair scan, written at +1 (Z_k at 66s+3+k)
            zt = zpool.tile([128, gc * 66 + 4], BF16, tag="zt")
            nc.vector.tensor_tensor_scan(
                zt[:, 1 : gc * 66 + 1], bias_m[:, : gc * 66], mt[:, : gc * 66], 0.0,
                mybir.AluOpType.add, mybir.AluOpType.max,
            )
            zts = zt[:, : gc * 66].rearrange("p (s e) -> p s e", s=gc)
            # fix: even-h outputs R_2k = max(Z_{k-1}, E_k); window starts even
            rt = rpool.tile([128, hw], BF16, tag="rt")
            rts = rt[:].rearrange("p (s e) -> p s e", s=gc)
            nc.vector.tensor_tensor(
                rts[:], zts[:, :, 2:66], btEv[:], mybir.AluOpType.max
            )
            # stores: evens from rt, odds (Z runs) from zt
            ovv = ov[:, s0 * H : s0 * H + fw].rearrange("p (s e) -> p s e", s=gc)
            nc.scalar.dma_start(out=ovv[:, :, 0:64], in_=rts[:])
            ztz = zt[:, 3 : 3 + gc * 66].rearrange("p (s e) -> p s e", s=gc)
            nc.gpsimd.dma_start(out=ovv[:, :, 64:128], in_=ztz[:, :, 0:64])
    nc.finalize()
    return nc


def kernel(x: np.ndarray) -> np.ndarray:
    global LAST_RESULTS
    import ml_dtypes

    assert x.shape == (B, C, H, W)
    xb = (np.asarray(x, dtype=np.float32) * 2.0).astype(ml_dtypes.bfloat16)
    xs = xb.reshape(N_CORES, S, H, W)
    in_maps = [
        {"x": np.ascontiguousarray(xs[i].transpose(1, 0, 2)).reshape(H, S * W)}
        for i in range(N_CORES)
    ]
    nc = build_nc(S, g=32, bufs=4, taper=2)
    res = run_bass_kernel_spmd(nc, in_maps, core_ids=list(range(N_CORES)))
    LAST_RESULTS = res
    # o[w, s*128 + (hE|hO)]: out[s, 2j+b, w] = o[w, s, b, j]
    out = np.empty((N_CORES, S, H, W), dtype=np.float32)
    for i in range(N_CORES):
        oi = np.asarray(res.results[i]["o"]).reshape(W, S, 2, 64).astype(np.float32)
        out[i, :, 0::2, :] = oi[:, :, 0, :].transpose(1, 2, 0)
        out[i, :, 1::2, :] = oi[:, :, 1, :].transpose(1, 2, 0)
    return out.reshape(B, C, H, W)


# revision 20
# speedup vs baseline: 1.0732x; 1.0070x over previous
"""Trainium2 Bass kernel: out = 2 * cummax_W(cummax_H(x)) for x [16,256,128,128] f32.

Precision: gate is rel_err < 2e-2; device works on xb = bf16(2*x) (host
downcast; x2 folded into the input -- exact since max/x2 commute and bf16*2 is
exact). Only error is the input rounding (~2^-9 relative).

The DVE scan (tensor_tensor_scan) runs at 2 cyc/elem and is the bottleneck;
TT-max on aligned contiguous bf16 runs at 0.5 cyc/elem. The H pass therefore
uses a pair-trick: pair-combine adjacent h rows (0.5), scan only the pair
maxima (half the elements), and rebuild even-h outputs with one more TT-max
(0.5) -- odd-h outputs are the scan result itself. The h-even/odd split is
produced by the scalar engine while staging PSUM->SBUF (strided reads there
are off the critical path). All DVE combine/fix APs keep the 2x_1P mode:
even strides, 4B-aligned starts, contiguous runs; the scan output lands at +1
in a 66-per-slice padded layout so the fix window starts even and sees NEG at
slice boundaries.

Per core (512 slices), per supertile of g=16 slices:
  1. DMA load xt [p=h, f=(s,w)] bf16 (host h-major; 4KB descriptors).
  2. DVE W-scan: segmented cummax (bias NEG at each slice's first column).
  3. PE transposes -> PSUM bf16 [p=w, f=(s,h)], 8 slices per bank tile.
  4. Scalar: deinterleaved staging PSUM->SBUF: btE (h even), btO (h odd).
  5. DVE: combine m=max(btE,btO) into padded mt; segmented pair scan -> zt@+1;
     fix rtH = max(zt window, btE).
  6. Stores: rtH (h-even results) and zt Z-runs (h-odd results) to a
     [w, s, hE|hO] DRAM layout; host re-interleaves and upcasts.
"""

from contextlib import ExitStack

import numpy as np

import concourse.bass as bass
import concourse.tile as tile
from concourse import bacc, mybir
from concourse.bass_utils import run_bass_kernel_spmd
from concourse.masks import make_identity

N_CORES = 8
B, C, H, W = 16, 256, 128, 128
S = (B // N_CORES) * C  # 512 slices per core
NEG = -3.0e38

F32 = mybir.dt.float32
BF16 = mybir.dt.bfloat16

LAST_RESULTS = None


def build_nc(n_slices: int = S, g: int = 16, bufs: int = 6, taper: int = 2) -> bass.Bass:
    nc = bacc.Bacc(None, target_bir_lowering=False)
    # h-major input: x[h, s*W + w]; output o[w, s*128 + (hE|hO)]
    x = nc.declare_dram_parameter("x", [H, n_slices * W], BF16, isOutput=False)
    o = nc.declare_dram_parameter("o", [W, n_slices * H], BF16, isOutput=True)

    gs = g // 2
    chunks = []
    pos = 0
    for _ in range(taper):
        chunks.append((pos, gs))
        pos += gs
    tail = n_slices - taper * gs
    while pos < tail:
        chunks.append((pos, g))
        pos += g
    for _ in range(taper):
        chunks.append((pos, gs))
        pos += gs
    assert pos == n_slices

    with ExitStack() as ctx:
        tc = ctx.enter_context(tile.TileContext(nc))
        consts = ctx.enter_context(tc.tile_pool(name="consts", bufs=1))
        ident = consts.tile([128, 128], BF16)
        make_identity(nc, ident)
        # W-scan bias over (s,w): NEG at each slice's first column
        bias = consts.tile([128, g * W], BF16)
        nc.vector.memset(bias, 0.0)
        for gi in range(g):
            nc.vector.memset(bias[:, gi * W : gi * W + 1], NEG)
        # pair-scan bias over padded m layout: NEG at each slice's first pad
        bias_m = consts.tile([128, g * 66], BF16)
        nc.vector.memset(bias_m, 0.0)
        for gi in range(g):
            nc.vector.memset(bias_m[:, gi * 66 : gi * 66 + 1], NEG)

        xpool = ctx.enter_context(tc.tile_pool(name="xt", bufs=bufs))
        apool = ctx.enter_context(tc.tile_pool(name="at", bufs=bufs))
        epool = ctx.enter_context(tc.tile_pool(name="be", bufs=bufs))
        opool = ctx.enter_context(tc.tile_pool(name="bo", bufs=bufs))
        mpool = ctx.enter_context(tc.tile_pool(name="mt", bufs=2))
        zpool = ctx.enter_context(tc.tile_pool(name="zt", bufs=bufs))
        rpool = ctx.enter_context(tc.tile_pool(name="rt", bufs=bufs))
        pa_pool = ctx.enter_context(tc.tile_pool(name="pa", bufs=6, space="PSUM"))

        xv = x.ap()
        ov = o.ap()

        for ci, (s0, gc) in enumerate(chunks):
            fw = gc * W
            xt = xpool.tile([128, fw], BF16, tag="xt")
            nc.sync.dma_start(out=xt[:], in_=xv[:, s0 * W : s0 * W + fw])
            at = apool.tile([128, fw], BF16, tag="at")
            nc.vector.tensor_tensor_scan(
                at[:], bias[:, :fw], xt[:], 0.0,
                mybir.AluOpType.add, mybir.AluOpType.max,
            )
            # transposes + deinterleaved scalar staging
            hw = gc * 64
            btE = epool.tile([128, hw], BF16, tag="be")
            btO = opool.tile([128, hw], BF16, tag="bo")
            btEv = btE[:].rearrange("p (s e) -> p s e", s=gc)
            btOv = btO[:].rearrange("p (s e) -> p s e", s=gc)
            for hb in range(gc // 8):
                pa = pa_pool.tile([128, 1024], BF16, tag="pa")
                for j in range(8):
                    s = hb * 8 + j
                    nc.tensor.transpose(
                        pa[:, j * 128 : (j + 1) * 128],
                        at[:, s * 128 : (s + 1) * 128],
                        ident[:],
                    )
                pav = pa[:].rearrange("p (s hj hb) -> p s hj hb", s=8, hb=2)
                nc.scalar.copy(btEv[:, hb * 8 : (hb + 1) * 8], pav[:, :, :, 0])
                nc.scalar.copy(btOv[:, hb * 8 : (hb + 1) * 8], pav[:, :, :, 1])
            # pair combine into padded m layout (full-width tile so the NEG
            # pads persist across the pool's buffer rotation)
            mt = mpool.tile([128, g * 66], BF16, tag="mt")
            mts = mt[:, : gc * 66].rearrange("p (s e) -> p s e", s=gc)
            if ci < 2:
                mfull = mt[:].rearrange("p (s e) -> p s e", s=g)
                nc.vector.memset(mfull[:, :, 0:2], NEG)
            nc.vector.tensor_tensor(
                mts[:, :, 2:66], btEv[:], btOv[:], mybir.AluOpType.max
            )
            # segmented pair scan, written at +1 (Z_k at 66s+3+k)
            zt = zpool.tile([128, gc * 66 + 4], BF16, tag="zt")
            nc.vector.tensor_tensor_scan(
                zt[:, 1 : gc * 66 + 1], bias_m[:, : gc * 66], mt[:, : gc * 66], 0.0,
                mybir.AluOpType.add, mybir.AluOpType.max,
            )
            zts = zt[:, : gc * 66].rearrange("p (s e) -> p s e", s=gc)
            # fix: even-h outputs R_2k = max(Z_{k-1}, E_k); window starts even
            rt = rpool.tile([128, hw], BF16, tag="rt")
            rts = rt[:].rearrange("p (s e) -> p s e", s=gc)
            nc.vector.tensor_tensor(
                rts[:], zts[:, :, 2:66], btEv[:], mybir.AluOpType.max
            )
            # stores: evens from rt, odds (Z runs) from zt
            ovv = ov[:, s0 * H : s0 * H + fw].rearrange("p (s e) -> p s e", s=gc)
            nc.scalar.dma_start(out=ovv[:, :, 0:64], in_=rts[:])
            ztz = zt[:, 3 : 3 + gc * 66].rearrange("p (s e) -> p s e", s=gc)
            nc.gpsimd.dma_start(out=ovv[:, :, 64:128], in_=ztz[:, :, 0:64])
    nc.finalize()
    return nc


def kernel(x: np.ndarray) -> np.ndarray:
    global LAST_RESULTS
    import ml_dtypes

    assert x.shape == (B, C, H, W)
    xb = (np.asarray(x, dtype=np.float32) * 2.0).astype(ml_dtypes.bfloat16)
    xs = xb.reshape(N_CORES, S, H, W)
    in_maps = [
        {"x": np.ascontiguousarray(xs[i].transpose(1, 0, 2)).reshape(H, S * W)}
        for i in range(N_CORES)
    ]
    nc = build_nc(S, g=16, bufs=6, taper=2)
    res = run_bass_kernel_spmd(nc, in_maps, core_ids=list(range(N_CORES)))
    LAST_RESULTS = res
    # o[w, s*128 + (hE|hO)]: out[s, 2j+b, w] = o[w, s, b, j]
    out = np.empty((N_CORES, S, H, W), dtype=np.float32)
    for i in range(N_CORES):
        oi = np.asarray(res.results[i]["o"]).reshape(W, S, 2, 64).astype(np.float32)
        out[i, :, 0::2, :] = oi[:, :, 0, :].transpose(1, 2, 0)
        out[i, :, 1::2, :] = oi[:, :, 1, :].transpose(1, 2, 0)
    return out.reshape(B, C, H, W)


# revision 21
# speedup vs baseline: 1.0772x; 1.0038x over previous
"""Trainium2 Bass kernel: out = 2 * cummax_W(cummax_H(x)) for x [16,256,128,128] f32.

Precision: gate is rel_err < 2e-2; device works on xb = bf16(2*x) (host
downcast; x2 folded into the input -- exact since max/x2 commute and bf16*2 is
exact). Only error is the input rounding (~2^-9 relative).

The DVE scan (tensor_tensor_scan) runs at 2 cyc/elem and is the bottleneck;
TT-max on aligned contiguous bf16 runs at 0.5 cyc/elem. The H pass therefore
uses a pair-trick: pair-combine adjacent h rows (0.5), scan only the pair
maxima (half the elements), and rebuild even-h outputs with one more TT-max
(0.5) -- odd-h outputs are the scan result itself. The h-even/odd split is
produced by the scalar engine while staging PSUM->SBUF (strided reads there
are off the critical path). All DVE combine/fix APs keep the 2x_1P mode:
even strides, 4B-aligned starts, contiguous runs; the scan output lands at +1
in a 66-per-slice padded layout so the fix window starts even and sees NEG at
slice boundaries.

Per core (512 slices), per supertile of g=16 slices:
  1. DMA load xt [p=h, f=(s,w)] bf16 (host h-major; 4KB descriptors).
  2. DVE W-scan: segmented cummax (bias NEG at each slice's first column).
  3. PE transposes -> PSUM bf16 [p=w, f=(s,h)], 8 slices per bank tile.
  4. Scalar: deinterleaved staging PSUM->SBUF: btE (h even), btO (h odd).
  5. DVE: combine m=max(btE,btO) into padded mt; segmented pair scan -> zt@+1;
     fix rtH = max(zt window, btE).
  6. Stores: rtH (h-even results) and zt Z-runs (h-odd results) to a
     [w, s, hE|hO] DRAM layout; host re-interleaves and upcasts.
"""

from contextlib import ExitStack

import numpy as np

import concourse.bass as bass
import concourse.tile as tile
from concourse import bacc, mybir
from concourse.bass_utils import run_bass_kernel_spmd
from concourse.masks import make_identity

N_CORES = 8
B, C, H, W = 16, 256, 128, 128
S = (B // N_CORES) * C  # 512 slices per core
NEG = -3.0e38

F32 = mybir.dt.float32
BF16 = mybir.dt.bfloat16

LAST_RESULTS = None


def build_nc(n_slices: int = S, g: int = 16, bufs: int = 6, taper: int = 2) -> bass.Bass:
    nc = bacc.Bacc(None, target_bir_lowering=False)
    # h-major input: x[h, s*W + w]; output o[w, s*128 + (hE|hO)]
    x = nc.declare_dram_parameter("x", [H, n_slices * W], BF16, isOutput=False)
    o = nc.declare_dram_parameter("o", [W, n_slices * H], BF16, isOutput=True)

    gs = g // 2
    chunks = []
    pos = 0
    for _ in range(taper):
        chunks.append((pos, gs))
        pos += gs
    tail = n_slices - taper * gs
    while pos < tail:
        chunks.append((pos, g))
        pos += g
    for _ in range(taper):
        chunks.append((pos, gs))
        pos += gs
    assert pos == n_slices

    with ExitStack() as ctx:
        tc = ctx.enter_context(tile.TileContext(nc))
        consts = ctx.enter_context(tc.tile_pool(name="consts", bufs=1))
        ident = consts.tile([128, 128], BF16)
        make_identity(nc, ident)
        # W-scan bias over (s,w): NEG at each slice's first column
        bias = consts.tile([128, g * W], BF16)
        nc.gpsimd.memset(bias, 0.0)
        for gi in range(g):
            nc.gpsimd.memset(bias[:, gi * W : gi * W + 1], NEG)
        # pair-scan bias over padded m layout: NEG at each slice's first pad
        bias_m = consts.tile([128, g * 66], BF16)
        nc.gpsimd.memset(bias_m, 0.0)
        for gi in range(g):
            nc.gpsimd.memset(bias_m[:, gi * 66 : gi * 66 + 1], NEG)

        xpool = ctx.enter_context(tc.tile_pool(name="xt", bufs=bufs))
        apool = ctx.enter_context(tc.tile_pool(name="at", bufs=bufs))
        epool = ctx.enter_context(tc.tile_pool(name="be", bufs=bufs))
        opool = ctx.enter_context(tc.tile_pool(name="bo", bufs=bufs))
        mpool = ctx.enter_context(tc.tile_pool(name="mt", bufs=2))
        zpool = ctx.enter_context(tc.tile_pool(name="zt", bufs=bufs))
        rpool = ctx.enter_context(tc.tile_pool(name="rt", bufs=bufs))
        pa_pool = ctx.enter_context(tc.tile_pool(name="pa", bufs=6, space="PSUM"))

        xv = x.ap()
        ov = o.ap()

        for ci, (s0, gc) in enumerate(chunks):
            fw = gc * W
            xt = xpool.tile([128, fw], BF16, tag="xt")
            nc.sync.dma_start(out=xt[:], in_=xv[:, s0 * W : s0 * W + fw])
            at = apool.tile([128, fw], BF16, tag="at")
            nc.vector.tensor_tensor_scan(
                at[:], bias[:, :fw], xt[:], 0.0,
                mybir.AluOpType.add, mybir.AluOpType.max,
            )
            # transposes + deinterleaved scalar staging
            hw = gc * 64
            btE = epool.tile([128, hw], BF16, tag="be")
            btO = opool.tile([128, hw], BF16, tag="bo")
            btEv = btE[:].rearrange("p (s e) -> p s e", s=gc)
            btOv = btO[:].rearrange("p (s e) -> p s e", s=gc)
            for hb in range(gc // 8):
                pa = pa_pool.tile([128, 1024], BF16, tag="pa")
                for j in range(8):
                    s = hb * 8 + j
                    nc.tensor.transpose(
                        pa[:, j * 128 : (j + 1) * 128],
                        at[:, s * 128 : (s + 1) * 128],
                        ident[:],
                    )
                pav = pa[:].rearrange("p (s hj hb) -> p s hj hb", s=8, hb=2)
                nc.scalar.copy(btEv[:, hb * 8 : (hb + 1) * 8], pav[:, :, :, 0])
                nc.scalar.copy(btOv[:, hb * 8 : (hb + 1) * 8], pav[:, :, :, 1])
            # pair combine into padded m layout (full-width tile so the NEG
            # pads persist across the pool's buffer rotation)
            mt = mpool.tile([128, g * 66], BF16, tag="mt")
            mts = mt[:, : gc * 66].rearrange("p (s e) -> p s e", s=gc)
            if ci < 2:
                mfull = mt[:].rearrange("p (s e) -> p s e", s=g)
                nc.gpsimd.memset(mfull[:, :, 0:2], NEG)
            nc.vector.tensor_tensor(
                mts[:, :, 2:66], btEv[:], btOv[:], mybir.AluOpType.max
            )
            # segmented pair scan, written at +1 (Z_k at 66s+3+k)
            zt = zpool.tile([128, gc * 66 + 4], BF16, tag="zt")
            nc.vector.tensor_tensor_scan(
                zt[:, 1 : gc * 66 + 1], bias_m[:, : gc * 66], mt[:, : gc * 66], 0.0,
                mybir.AluOpType.add, mybir.AluOpType.max,
            )
            zts = zt[:, : gc * 66].rearrange("p (s e) -> p s e", s=gc)
            # fix: even-h outputs R_2k = max(Z_{k-1}, E_k); window starts even
            rt = rpool.tile([128, hw], BF16, tag="rt")
            rts = rt[:].rearrange("p (s e) -> p s e", s=gc)
            nc.vector.tensor_tensor(
                rts[:], zts[:, :, 2:66], btEv[:], mybir.AluOpType.max
            )
            # stores: evens from rt, odds (Z runs) from zt
            ovv = ov[:, s0 * H : s0 * H + fw].rearrange("p (s e) -> p s e", s=gc)
            nc.scalar.dma_start(out=ovv[:, :, 0:64], in_=rts[:])
            ztz = zt[:, 3 : 3 + gc * 66].rearrange("p (s e) -> p s e", s=gc)
            nc.gpsimd.dma_start(out=ovv[:, :, 64:128], in_=ztz[:, :, 0:64])
    nc.finalize()
    return nc


def kernel(x: np.ndarray) -> np.ndarray:
    global LAST_RESULTS
    import ml_dtypes

    assert x.shape == (B, C, H, W)
    xb = (np.asarray(x, dtype=np.float32) * 2.0).astype(ml_dtypes.bfloat16)
    xs = xb.reshape(N_CORES, S, H, W)
    in_maps = [
        {"x": np.ascontiguousarray(xs[i].transpose(1, 0, 2)).reshape(H, S * W)}
        for i in range(N_CORES)
    ]
    nc = build_nc(S, g=16, bufs=6, taper=2)
    res = run_bass_kernel_spmd(nc, in_maps, core_ids=list(range(N_CORES)))
    LAST_RESULTS = res
    # o[w, s*128 + (hE|hO)]: out[s, 2j+b, w] = o[w, s, b, j]
    out = np.empty((N_CORES, S, H, W), dtype=np.float32)
    for i in range(N_CORES):
        oi = np.asarray(res.results[i]["o"]).reshape(W, S, 2, 64).astype(np.float32)
        out[i, :, 0::2, :] = oi[:, :, 0, :].transpose(1, 2, 0)
        out[i, :, 1::2, :] = oi[:, :, 1, :].transpose(1, 2, 0)
    return out.reshape(B, C, H, W)
